# revision 16
# baseline (speedup 1.0000x reference)
"""EnhancedMACDCell forward on 8 Trainium2 NeuronCores.

The reference computes, per batch row b of price_series [B, 64]:
    macd[b, j]  = w_fast . price[b, e-12:e] - w_slow . price[b, e-26:e]
                  + (b_fast - b_slow),        e = 64 - 8 + j, j = 0..8
    signal[b]   = w_sig . macd[b, :] + b_sig
    hist[b]     = macd[b, 8] - signal[b]
    out[b]      = tanh(hist[b] * norm_scale + norm_bias)

Everything before the tanh is linear in price_series, so the whole model
collapses to a single 64-tap linear functional per row:
    out[b] = tanh(price[b, :] . u + c0)
with u / c0 computed on the host (float64) from the tiny weight inputs.
Only columns 30..63 of u are nonzero, so the on-device compute is a
34-wide weighted row reduction + tanh over [1M, 64] float32 - purely
memory bound (32 MiB of HBM reads per core).

Sharding: pure data parallel - 8 equal batch shards, weights replicated.
"""

import os
import sys

import numpy as np

for _p in ("/opt/trn_rl_repo", "/root/.axon_site/_ro/trn_rl_repo"):
    if os.path.isdir(_p) and _p not in sys.path:
        sys.path.insert(0, _p)

import concourse.bacc as bacc
import concourse.bass as bass
import concourse.mybir as mybir
from concourse import tile
from concourse.bass_utils import run_bass_kernel_spmd

FAST, SLOW, SIG = 12, 26, 9
S = 64
N_CORES = 8
P = 128           # SBUF partitions
R = 64            # batch rows packed per partition per tile
C_LO, C_HI = 30, 64
C = C_HI - C_LO   # 34 columns with nonzero weight


def _collapsed_weights(w_fast, b_fast, w_slow, b_slow, w_sig, b_sig,
                       norm_scale, norm_bias):
    """Fold the whole linear pipeline into (u[64], c0)."""
    wf = np.asarray(w_fast, np.float64).reshape(-1)
    ws = np.asarray(w_slow, np.float64).reshape(-1)
    wg = np.asarray(w_sig, np.float64).reshape(-1)
    A = np.zeros((SIG, S), np.float64)
    for j in range(SIG):
        e = S - (SIG - 1) + j
        A[j, e - FAST:e] += wf
        A[j, e - SLOW:e] -= ws
    coeff = -wg.copy()
    coeff[SIG - 1] += 1.0
    u = coeff @ A
    c0 = (float(np.asarray(b_fast).reshape(-1)[0])
          - float(np.asarray(b_slow).reshape(-1)[0])) * coeff.sum() \
        - float(np.asarray(b_sig).reshape(-1)[0])
    ns = float(np.asarray(norm_scale).reshape(-1)[0])
    nb = float(np.asarray(norm_bias).reshape(-1)[0])
    return (u * ns).astype(np.float32), float(c0 * ns + nb)


def _tile_schedule(total_r: int, r_max: int = 128):
    """Tile sizes (in rows-per-partition units): small at the start so DVE
    can begin early, small at the end to shorten the critical tail."""
    head = [16, 16, 32, 64]
    tail = [64, 32, 16, 16]
    mid_r = total_r - sum(head) - sum(tail)
    assert mid_r >= 0 and mid_r % r_max == 0
    return head + [r_max] * (mid_r // r_max) + tail


def _build_v3(b_core: int, c0: float, bufs: int = 4) -> bass.Bass:
    nc = bacc.Bacc()
    x = nc.declare_dram_parameter("x", [b_core, S], mybir.dt.float32,
                                  isOutput=False)
    w = nc.declare_dram_parameter("w", [P, C], mybir.dt.float32,
                                  isOutput=False)
    y = nc.declare_dram_parameter("y", [b_core], mybir.dt.float32,
                                  isOutput=True)

    total_r = b_core // P
    sched = _tile_schedule(total_r)

    with tile.TileContext(nc) as tc:
        with (
            tc.tile_pool(name="wp", bufs=1) as wp,
            tc.tile_pool(name="xp", bufs=bufs) as xp,
            tc.tile_pool(name="pp", bufs=2) as pp,
            tc.tile_pool(name="rp", bufs=2) as rp,
            tc.tile_pool(name="op", bufs=2) as op,
        ):
            wt = wp.tile([P, C], mybir.dt.float32)
            nc.gpsimd.dma_start(wt[:], w[:])
            bt = wp.tile([P, 1], mybir.dt.float32, tag="bias")
            nc.vector.memset(bt[:], c0)
            base = 0
            for i, ri in enumerate(sched):
                rows = P * ri
                xvi = x[base:base + rows, :].rearrange("(p r) s -> p r s", p=P)
                yvi = y[base:base + rows].rearrange("(p r) -> p r", p=P)
                dma_eng = nc.scalar if i % 2 else nc.sync
                xt = xp.tile([P, ri * S], mybir.dt.float32)
                x3full = xt[:].rearrange("p (r s) -> p r s", s=S)
                dma_eng.dma_start(x3full, xvi)
                x3 = x3full[:, :, C_LO:C_HI]
                pt = pp.tile([P, ri * C], mybir.dt.float32)
                p3 = pt[:].rearrange("p (r c) -> p r c", c=C)
                wb = wt[:].unsqueeze(1).broadcast_to([P, ri, C])
                nc.vector.tensor_mul(p3, x3, wb)
                rt = rp.tile([P, ri], mybir.dt.float32)
                nc.vector.reduce_sum(rt[:], p3, axis=mybir.AxisListType.X)
                ot = op.tile([P, ri], mybir.dt.float32)
                nc.scalar.activation(ot[:], rt[:],
                                     mybir.ActivationFunctionType.Tanh,
                                     bias=bt[:, 0:1], scale=1.0)
                nc.gpsimd.dma_start(yvi, ot[:])
                base += rows
    nc.compile()
    return nc


def _build_v4(b_core: int, c0: float, bufs: int = 4,
              head=(16, 16, 32, 64), tail=(64, 32, 16, 16),
              r_max: int = 128) -> bass.Bass:
    """Variable-size loads inside p-major uniform blocks of r_max rows per
    partition; all outputs accumulate in one SBUF tile, flushed by two
    large aligned DMAs. Input loads alternate between the two HWDGE rings
    and are the only traffic during the stream."""
    nc = bacc.Bacc()
    x = nc.declare_dram_parameter("x", [b_core, S], mybir.dt.float32,
                                  isOutput=False)
    w = nc.declare_dram_parameter("w", [P, C], mybir.dt.float32,
                                  isOutput=False)
    y = nc.declare_dram_parameter("y", [b_core], mybir.dt.float32,
                                  isOutput=True)

    total_r = b_core // P
    n_blocks = total_r // r_max
    assert total_r % r_max == 0
    assert sum(head) == r_max and sum(tail) == r_max

    # chunks: (block, off, ri)
    chunks = []
    for off, ri in zip(np.cumsum((0,) + head[:-1]), head):
        chunks.append((0, int(off), ri))
    for n in range(1, n_blocks - 1):
        chunks.append((n, 0, r_max))
    for off, ri in zip(np.cumsum((0,) + tail[:-1]), tail):
        chunks.append((n_blocks - 1, int(off), ri))

    xb = x[:].rearrange("(n p r) s -> n p r s", p=P, r=r_max)
    yb = y[:].rearrange("(n p r) -> p n r", p=P, r=r_max)

    with tile.TileContext(nc) as tc:
        with (
            tc.tile_pool(name="wp", bufs=1) as wp,
            tc.tile_pool(name="xp", bufs=bufs) as xp,
            tc.tile_pool(name="pp", bufs=2) as pp,
            tc.tile_pool(name="rp", bufs=2) as rp,
            tc.tile_pool(name="op", bufs=1) as op,
        ):
            wt = wp.tile([P, C], mybir.dt.float32)
            nc.sync.dma_start(wt[:], w[:])
            bt = wp.tile([P, 1], mybir.dt.float32, tag="bias")
            nc.vector.memset(bt[:], c0)
            ot = op.tile([P, total_r], mybir.dt.float32)

            last_mid_act = None
            for i, (n, off, ri) in enumerate(chunks):
                dma_eng = nc.scalar if i % 2 else nc.sync
                xt = xp.tile([P, ri * S], mybir.dt.float32)
                x3 = xt[:].rearrange("p (r s) -> p r s", s=S)
                dma_eng.dma_start(x3, xb[n][:, off:off + ri, :])
                pt = pp.tile([P, ri * C], mybir.dt.float32)
                p3 = pt[:].rearrange("p (r c) -> p r c", c=C)
                wb = wt[:].unsqueeze(1).broadcast_to([P, ri, C])
                nc.vector.tensor_mul(p3, x3[:, :, C_LO:C_HI], wb)
                rt = rp.tile([P, ri], mybir.dt.float32)
                nc.vector.reduce_sum(rt[:], p3, axis=mybir.AxisListType.X)
                col = n * r_max + off
                nc.scalar.activation(ot[:, col:col + ri], rt[:],
                                     mybir.ActivationFunctionType.Tanh,
                                     bias=bt[:, 0:1], scale=1.0)
                if n == n_blocks - 2 and off + ri == r_max:
                    # all blocks except the last are now computed: flush them
                    o3 = ot[:, :(n_blocks - 1) * r_max].rearrange(
                        "p (n r) -> p n r", r=r_max)
                    nc.sync.dma_start(yb[:, :n_blocks - 1, :], o3)
            o3t = ot[:, (n_blocks - 1) * r_max:].rearrange(
                "p (n r) -> p n r", r=r_max)
            nc.sync.dma_start(yb[:, n_blocks - 1:, :], o3t)
    nc.compile()
    return nc


def _build_v5(b_core: int, c0: float, bufs: int = 4,
              head=(32, 96), tail=(64, 32, 32),
              r_max: int = 128, split_loads: bool = True) -> bass.Bass:
    """v4 + every load split across both HWDGE rings; strict DVE ordering
    for the tail chunks (pp bufs=1)."""
    nc = bacc.Bacc()
    x = nc.declare_dram_parameter("x", [b_core, S], mybir.dt.float32,
                                  isOutput=False)
    w = nc.declare_dram_parameter("w", [P, C], mybir.dt.float32,
                                  isOutput=False)
    y = nc.declare_dram_parameter("y", [b_core], mybir.dt.float32,
                                  isOutput=True)

    total_r = b_core // P
    n_blocks = total_r // r_max
    assert total_r % r_max == 0
    assert sum(head) == r_max and sum(tail) == r_max

    chunks = []
    for off, ri in zip(np.cumsum((0,) + head[:-1]), head):
        chunks.append((0, int(off), ri))
    for n in range(1, n_blocks - 1):
        chunks.append((n, 0, r_max))
    for off, ri in zip(np.cumsum((0,) + tail[:-1]), tail):
        chunks.append((n_blocks - 1, int(off), ri))
    n_tail = len(tail)

    xb = x[:].rearrange("(n p r) s -> n p r s", p=P, r=r_max)
    yb = y[:].rearrange("(n p r) -> p n r", p=P, r=r_max)

    with tile.TileContext(nc) as tc:
        with (
            tc.tile_pool(name="wp", bufs=1) as wp,
            tc.tile_pool(name="xp", bufs=bufs) as xp,
            tc.tile_pool(name="pp", bufs=2) as pp,
            tc.tile_pool(name="ppt", bufs=1) as ppt,
            tc.tile_pool(name="rp", bufs=2) as rp,
            tc.tile_pool(name="op", bufs=1) as op,
        ):
            wt = wp.tile([P, C], mybir.dt.float32)
            nc.sync.dma_start(wt[:], w[:])
            bt = wp.tile([P, 1], mybir.dt.float32, tag="bias")
            nc.vector.memset(bt[:], c0)
            ot = op.tile([P, total_r], mybir.dt.float32)

            for i, (n, off, ri) in enumerate(chunks):
                xt = xp.tile([P, ri * S], mybir.dt.float32)
                x3 = xt[:].rearrange("p (r s) -> p r s", s=S)
                if split_loads and ri >= 2:
                    h = ri // 2
                    nc.sync.dma_start(x3[:, :h, :], xb[n][:, off:off + h, :])
                    nc.scalar.dma_start(x3[:, h:, :],
                                        xb[n][:, off + h:off + ri, :])
                else:
                    eng = nc.scalar if i % 2 else nc.sync
                    eng.dma_start(x3, xb[n][:, off:off + ri, :])
                pool = ppt if i >= len(chunks) - n_tail else pp
                pt = pool.tile([P, ri * C], mybir.dt.float32, tag="prod")
                p3 = pt[:].rearrange("p (r c) -> p r c", c=C)
                wb = wt[:].unsqueeze(1).broadcast_to([P, ri, C])
                nc.vector.tensor_mul(p3, x3[:, :, C_LO:C_HI], wb)
                rt = rp.tile([P, ri], mybir.dt.float32)
                nc.vector.reduce_sum(rt[:], p3, axis=mybir.AxisListType.X)
                col = n * r_max + off
                nc.scalar.activation(ot[:, col:col + ri], rt[:],
                                     mybir.ActivationFunctionType.Tanh,
                                     bias=bt[:, 0:1], scale=1.0)
                if n == n_blocks - 2 and off + ri == r_max:
                    o3 = ot[:, :(n_blocks - 1) * r_max].rearrange(
                        "p (n r) -> p n r", r=r_max)
                    nc.sync.dma_start(yb[:, :n_blocks - 1, :], o3)
            o3t = ot[:, (n_blocks - 1) * r_max:].rearrange(
                "p (n r) -> p n r", r=r_max)
            nc.sync.dma_start(yb[:, n_blocks - 1:, :], o3t)
    nc.compile()
    return nc


def _build_v6(b_core: int, c0: float, bufs: int = 4,
              head=(32, 96), tail=(64, 32, 32),
              r_max: int = 128, gps_mult: bool = True) -> bass.Bass:
    """Tile pipeline with GpSimd doing the multiplies for the mid blocks
    (DVE keeps all reduces + head/tail multiplies), a full-width result
    tile, and two batched tanh ACTs + flushes."""
    nc = bacc.Bacc()
    x = nc.declare_dram_parameter("x", [b_core, S], mybir.dt.float32,
                                  isOutput=False)
    w = nc.declare_dram_parameter("w", [P, C], mybir.dt.float32,
                                  isOutput=False)
    y = nc.declare_dram_parameter("y", [b_core], mybir.dt.float32,
                                  isOutput=True)

    total_r = b_core // P
    n_blocks = total_r // r_max
    assert total_r % r_max == 0
    assert sum(head) == r_max and sum(tail) == r_max

    chunks = []
    for off, ri in zip(np.cumsum((0,) + head[:-1]), head):
        chunks.append((0, int(off), ri))
    for n in range(1, n_blocks - 1):
        chunks.append((n, 0, r_max))
    for off, ri in zip(np.cumsum((0,) + tail[:-1]), tail):
        chunks.append((n_blocks - 1, int(off), ri))

    xb = x[:].rearrange("(n p r) s -> n p r s", p=P, r=r_max)
    yb = y[:].rearrange("(n p r) -> p n r", p=P, r=r_max)
    mid_r = (n_blocks - 1) * r_max

    with tile.TileContext(nc) as tc:
        with (
            tc.tile_pool(name="wp", bufs=1) as wp,
            tc.tile_pool(name="xp", bufs=bufs) as xp,
            tc.tile_pool(name="pp", bufs=2) as pp,
            tc.tile_pool(name="rp", bufs=1) as rp,
            tc.tile_pool(name="op", bufs=1) as op,
        ):
            wt = wp.tile([P, C], mybir.dt.float32)
            nc.sync.dma_start(wt[:], w[:])
            bt = wp.tile([P, 1], mybir.dt.float32, tag="bias")
            nc.vector.memset(bt[:], c0)
            rt = rp.tile([P, total_r], mybir.dt.float32)
            ot = op.tile([P, total_r], mybir.dt.float32)

            for i, (n, off, ri) in enumerate(chunks):
                is_mid = (0 < n < n_blocks - 1)
                eng = nc.scalar if i % 2 else nc.sync
                xt = xp.tile([P, ri * S], mybir.dt.float32)
                x3 = xt[:].rearrange("p (r s) -> p r s", s=S)
                eng.dma_start(x3, xb[n][:, off:off + ri, :])
                pt = pp.tile([P, ri * C], mybir.dt.float32, tag="prod")
                p3 = pt[:].rearrange("p (r c) -> p r c", c=C)
                wb = wt[:].unsqueeze(1).broadcast_to([P, ri, C])
                mul_eng = nc.gpsimd if (gps_mult and is_mid) else nc.vector
                mul_eng.tensor_mul(p3, x3[:, :, C_LO:C_HI], wb)
                col = n * r_max + off
                nc.vector.reduce_sum(rt[:, col:col + ri], p3,
                                     axis=mybir.AxisListType.X)
                if n == n_blocks - 2 and off + ri == r_max:
                    nc.scalar.activation(ot[:, :mid_r], rt[:, :mid_r],
                                         mybir.ActivationFunctionType.Tanh,
                                         bias=bt[:, 0:1], scale=1.0)
                    o3 = ot[:, :mid_r].rearrange("p (n r) -> p n r", r=r_max)
                    nc.sync.dma_start(yb[:, :n_blocks - 1, :], o3)
            nc.scalar.activation(ot[:, mid_r:], rt[:, mid_r:],
                                 mybir.ActivationFunctionType.Tanh,
                                 bias=bt[:, 0:1], scale=1.0)
            o3t = ot[:, mid_r:].rearrange("p (n r) -> p n r", r=r_max)
            nc.sync.dma_start(yb[:, n_blocks - 1:, :], o3t)
    nc.compile()
    return nc


def _build_v7(b_core: int, c0: float, bufs: int = 4,
              head=(32, 96), tail=(64, 32, 32),
              r_max: int = 128, gps_mult: bool = False) -> bass.Bass:
    """v5 pipeline but each DMA descriptor covers a ROW PAIR sliced to
    [row b cols 30:64 | row b+1 cols 0:64] = 98 f32 = 392 B contiguous,
    cutting HBM read bytes by 23%. Each chunk then needs two strided
    multiplies + two strided reduces (even/odd rows)."""
    nc = bacc.Bacc()
    x = nc.declare_dram_parameter("x", [b_core, S], mybir.dt.float32,
                                  isOutput=False)
    w = nc.declare_dram_parameter("w", [P, C], mybir.dt.float32,
                                  isOutput=False)
    y = nc.declare_dram_parameter("y", [b_core], mybir.dt.float32,
                                  isOutput=True)

    total_r = b_core // P
    n_blocks = total_r // r_max
    assert sum(head) == r_max and sum(tail) == r_max

    chunks = []
    for off, ri in zip(np.cumsum((0,) + head[:-1]), head):
        chunks.append((0, int(off), ri))
    for n in range(1, n_blocks - 1):
        chunks.append((n, 0, r_max))
    for off, ri in zip(np.cumsum((0,) + tail[:-1]), tail):
        chunks.append((n_blocks - 1, int(off), ri))

    PR = 98  # f32 elems per row-pair descriptor
    # x as flat elems per partition-block: row (n, p, r) starts at elem
    # ((n*P + p)*r_max + r) * S ; pair descriptor starts at col C_LO of
    # even row r: offset (...)*S + C_LO, length 98.
    xf = x[:].rearrange("(n p r) s -> n p (r s)", p=P, r=r_max)
    yb = y[:].rearrange("(n p r) -> p n r", p=P, r=r_max)
    mid_r = (n_blocks - 1) * r_max

    with tile.TileContext(nc) as tc:
        with (
            tc.tile_pool(name="wp", bufs=1) as wp,
            tc.tile_pool(name="xp", bufs=bufs) as xp,
            tc.tile_pool(name="pp", bufs=2) as pp,
            tc.tile_pool(name="rp", bufs=1) as rp,
            tc.tile_pool(name="op", bufs=1) as op,
        ):
            wt = wp.tile([P, C], mybir.dt.float32)
            nc.sync.dma_start(wt[:], w[:])
            bt = wp.tile([P, 1], mybir.dt.float32, tag="bias")
            nc.vector.memset(bt[:], c0)
            rt = rp.tile([P, total_r], mybir.dt.float32)
            ot = op.tile([P, total_r], mybir.dt.float32)

            for i, (n, off, ri) in enumerate(chunks):
                assert ri % 2 == 0
                npairs = ri // 2
                eng = nc.scalar if i % 2 else nc.sync
                xt = xp.tile([P, npairs * PR], mybir.dt.float32)
                x3 = xt[:].rearrange("p (q e) -> p q e", e=PR)
                # DRAM view: row pairs of 128 elems, inner-sliced to
                # [C_LO : C_LO+98] -> 392 B contiguous per descriptor
                src = xf[n][:, off * S:(off + ri) * S]
                src = src.rearrange("p (q e) -> p q e", e=2 * S)
                eng.dma_start(x3, src[:, :, C_LO:C_LO + PR])
                pt = pp.tile([P, ri * C], mybir.dt.float32, tag="prod")
                p3 = pt[:].rearrange("p (r c) -> p r c", c=C)
                p4 = pt[:].rearrange("p (q two c) -> p q two c", two=2, c=C)
                wb = wt[:].unsqueeze(1).broadcast_to([P, npairs, C])
                mul_eng = nc.gpsimd if (gps_mult and 0 < n < n_blocks - 1) \
                    else nc.vector
                # pair layout: elems 0:34 = row b cols 30:64;
                # elems 34:98 = row b+1 cols 0:64 -> needed: 64:98
                mul_eng.tensor_mul(p4[:, :, 0, :], x3[:, :, 0:C], wb)
                mul_eng.tensor_mul(p4[:, :, 1, :], x3[:, :, 64:64 + C], wb)
                col = n * r_max + off
                nc.vector.reduce_sum(rt[:, col:col + ri], p3,
                                     axis=mybir.AxisListType.X)
                if n == n_blocks - 2 and off + ri == r_max:
                    nc.scalar.activation(ot[:, :mid_r], rt[:, :mid_r],
                                         mybir.ActivationFunctionType.Tanh,
                                         bias=bt[:, 0:1], scale=1.0)
                    o3 = ot[:, :mid_r].rearrange("p (n r) -> p n r", r=r_max)
                    nc.sync.dma_start(yb[:, :n_blocks - 1, :], o3)
            nc.scalar.activation(ot[:, mid_r:], rt[:, mid_r:],
                                 mybir.ActivationFunctionType.Tanh,
                                 bias=bt[:, 0:1], scale=1.0)
            o3t = ot[:, mid_r:].rearrange("p (n r) -> p n r", r=r_max)
            nc.sync.dma_start(yb[:, n_blocks - 1:, :], o3t)
    nc.compile()
    return nc


def _build_raw(b_core: int, c0: float, bufs: int = 4,
               head=(32, 96), tail=(64, 32, 32),
               r_max: int = 128) -> bass.Bass:
    """Raw bacc (no TileContext): hand-placed semaphores, no end-of-kernel
    barrier butterfly. Sync ring: even-chunk loads + output flushes.
    Scalar ring: odd-chunk loads + the two batched tanh ACTs.
    Vector: all multiplies + reduces in strict chunk order."""
    from contextlib import ExitStack

    nc = bacc.Bacc()
    x = nc.declare_dram_parameter("x", [b_core, S], mybir.dt.float32,
                                  isOutput=False)
    w = nc.declare_dram_parameter("w", [P, C], mybir.dt.float32,
                                  isOutput=False)
    y = nc.declare_dram_parameter("y", [b_core], mybir.dt.float32,
                                  isOutput=True)

    total_r = b_core // P
    n_blocks = total_r // r_max
    assert total_r % r_max == 0
    assert sum(head) == r_max and sum(tail) == r_max

    chunks = []
    for off, ri in zip(np.cumsum((0,) + head[:-1]), head):
        chunks.append((0, int(off), ri))
    for n in range(1, n_blocks - 1):
        chunks.append((n, 0, r_max))
    for off, ri in zip(np.cumsum((0,) + tail[:-1]), tail):
        chunks.append((n_blocks - 1, int(off), ri))
    n_chunks = len(chunks)
    n_premid = len(head) + (n_blocks - 2)  # chunks covering blocks 0..n-2

    xb = x[:].rearrange("(n p r) s -> n p r s", p=P, r=r_max)
    yb = y[:].rearrange("(n p r) -> p n r", p=P, r=r_max)
    mid_r = (n_blocks - 1) * r_max

    with ExitStack() as ctx:
        ef = ctx.enter_context
        xs = [ef(nc.sbuf_tensor(f"xs{k}", [P, r_max * S], mybir.dt.float32))
              for k in range(bufs)]
        pts = [ef(nc.sbuf_tensor(f"pt{k}", [P, r_max * C], mybir.dt.float32))
               for k in range(2)]
        rt = ef(nc.sbuf_tensor("rt", [P, total_r], mybir.dt.float32))
        ot = ef(nc.sbuf_tensor("ot", [P, total_r], mybir.dt.float32))
        wt = ef(nc.sbuf_tensor("wt", [P, C], mybir.dt.float32))
        bt = ef(nc.sbuf_tensor("bt", [P, 1], mybir.dt.float32))
        s_slot = [ef(nc.semaphore(f"s_slot{k}")) for k in range(bufs)]
        s_w = ef(nc.semaphore("s_w"))
        s_red = ef(nc.semaphore("s_red"))
        s_act = ef(nc.semaphore("s_act"))
        s_out = ef(nc.semaphore("s_out"))
        block = ef(nc.Block())

        def x_view(i):
            n, off, ri = chunks[i]
            slot = xs[i % bufs]
            return (slot[:, :ri * S].rearrange("p (r s) -> p r s", s=S),
                    xb[n][:, off:off + ri, :])

        @block.sync
        def _(sync):
            sync.dma_start(wt[:, :], w[:]).then_inc(s_w, 16)
            for i in range(0, n_chunks, 2):
                if i - bufs >= 0:
                    sync.wait_ge(s_red, i - bufs + 1)
                dst, src = x_view(i)
                sync.dma_start(dst, src).then_inc(s_slot[i % bufs], 16)
            sync.wait_ge(s_act, 1)
            o3 = ot[:, :mid_r].rearrange("p (n r) -> p n r", r=r_max)
            sync.dma_start(yb[:, :n_blocks - 1, :], o3).then_inc(s_out, 16)
            sync.wait_ge(s_act, 2)
            o3t = ot[:, mid_r:].rearrange("p (n r) -> p n r", r=r_max)
            sync.dma_start(yb[:, n_blocks - 1:, :], o3t).then_inc(s_out, 16)
            sync.wait_ge(s_out, 32)

        @block.scalar
        def _(act):
            for i in range(1, n_chunks, 2):
                if i - bufs >= 0:
                    act.wait_ge(s_red, i - bufs + 1)
                dst, src = x_view(i)
                act.dma_start(dst, src).then_inc(s_slot[i % bufs], 16)
            act.wait_ge(s_red, n_premid)
            nc.scalar.activation(ot[:, :mid_r], rt[:, :mid_r],
                                 mybir.ActivationFunctionType.Tanh,
                                 bias=bt[:, 0:1], scale=1.0
                                 ).then_inc(s_act, 1)
            act.wait_ge(s_red, n_chunks)
            nc.scalar.activation(ot[:, mid_r:], rt[:, mid_r:],
                                 mybir.ActivationFunctionType.Tanh,
                                 bias=bt[:, 0:1], scale=1.0
                                 ).then_inc(s_act, 1)

        @block.vector
        def _(vec):
            vec.memset(bt[:, :], c0)
            for i, (n, off, ri) in enumerate(chunks):
                if i == 0:
                    vec.wait_ge(s_w, 16)
                vec.wait_ge(s_slot[i % bufs], 16 * (i // bufs + 1))
                x3, _ = x_view(i)
                pt = pts[i % 2]
                p3 = pt[:, :ri * C].rearrange("p (r c) -> p r c", c=C)
                wb = wt[:, :].unsqueeze(1).broadcast_to([P, ri, C])
                nc.vector.tensor_mul(p3, x3[:, :, C_LO:C_HI], wb)
                col = n * r_max + off
                nc.vector.reduce_sum(rt[:, col:col + ri], p3,
                                     axis=mybir.AxisListType.X
                                     ).then_inc(s_red, 1)

    nc.compile()
    return nc


def _build_v8(b_core: int, c0: float, bufs: int = 4,
              head=(16, 16, 32, 64), tail=(64, 32, 16, 16),
              r_max: int = 128) -> bass.Bass:
    """fp16 pipeline on host-presliced input x[b_core, 34] (only the 34
    columns with nonzero collapsed weight, cast to fp16 on the host).
    GpSimd does the broadcast multiplies, DVE the segmented reduces
    (fp32 out), ACT the two batched tanh's, Sync/Scalar queues alternate
    the input loads. Output y is [128, total_r] fp16, partition-major so
    each flush is one contiguous descriptor per partition; the host
    unscrambles."""
    from contextlib import ExitStack

    nc = bacc.Bacc()
    x = nc.declare_dram_parameter("x", [b_core, C], mybir.dt.float16,
                                  isOutput=False)
    w = nc.declare_dram_parameter("w", [P, C], mybir.dt.float16,
                                  isOutput=False)
    total_r = b_core // P
    y = nc.declare_dram_parameter("y", [P, total_r], mybir.dt.float16,
                                  isOutput=True)

    n_blocks = total_r // r_max
    assert total_r % r_max == 0
    assert sum(head) == r_max and sum(tail) == r_max

    chunks = []
    for off, ri in zip(np.cumsum((0,) + head[:-1]), head):
        chunks.append((0, int(off), ri))
    for n in range(1, n_blocks - 1):
        chunks.append((n, 0, r_max))
    for off, ri in zip(np.cumsum((0,) + tail[:-1]), tail):
        chunks.append((n_blocks - 1, int(off), ri))
    n_chunks = len(chunks)
    n_premid = len(head) + (n_blocks - 2)  # chunks covering blocks 0..n-2

    xb = x[:].rearrange("(n p r) s -> n p r s", p=P, r=r_max)
    mid_r = (n_blocks - 1) * r_max

    with ExitStack() as ctx:
        ef = ctx.enter_context
        xs = [ef(nc.sbuf_tensor(f"xs{k}", [P, r_max * C], mybir.dt.float16))
              for k in range(bufs)]
        pts = [ef(nc.sbuf_tensor(f"pt{k}", [P, r_max * C], mybir.dt.float16))
               for k in range(2)]
        rt = ef(nc.sbuf_tensor("rt", [P, total_r], mybir.dt.float32))
        ot = ef(nc.sbuf_tensor("ot", [P, total_r], mybir.dt.float16))
        wt = ef(nc.sbuf_tensor("wt", [P, C], mybir.dt.float16))
        bt = ef(nc.sbuf_tensor("bt", [P, 1], mybir.dt.float32))
        s_slot = [ef(nc.semaphore(f"s_slot{k}")) for k in range(bufs)]
        s_w = ef(nc.semaphore("s_w"))
        s_mul = ef(nc.semaphore("s_mul"))
        s_red = ef(nc.semaphore("s_red"))
        s_act = ef(nc.semaphore("s_act"))
        s_out = ef(nc.semaphore("s_out"))
        block = ef(nc.Block())

        def x_view(i):
            n, off, ri = chunks[i]
            slot = xs[i % bufs]
            return (slot[:, :ri * C].rearrange("p (r s) -> p r s", s=C),
                    xb[n][:, off:off + ri, :])

        @block.sync
        def _(sync):
            sync.dma_start(wt[:, :], w[:]).then_inc(s_w, 16)
            for i in range(0, n_chunks, 2):
                if i - bufs >= 0:
                    sync.wait_ge(s_mul, i - bufs + 1)
                dst, src = x_view(i)
                sync.dma_start(dst, src).then_inc(s_slot[i % bufs], 16)
            sync.wait_ge(s_act, 1)
            sync.dma_start(y[:, :mid_r], ot[:, :mid_r]).then_inc(s_out, 16)
            sync.wait_ge(s_act, 2)
            sync.dma_start(y[:, mid_r:], ot[:, mid_r:]).then_inc(s_out, 16)
            sync.wait_ge(s_out, 32)

        @block.scalar
        def _(act):
            for i in range(1, n_chunks, 2):
                if i - bufs >= 0:
                    act.wait_ge(s_mul, i - bufs + 1)
                dst, src = x_view(i)
                act.dma_start(dst, src).then_inc(s_slot[i % bufs], 16)
            act.wait_ge(s_red, n_premid)
            nc.scalar.activation(ot[:, :mid_r], rt[:, :mid_r],
                                 mybir.ActivationFunctionType.Tanh,
                                 bias=bt[:, 0:1], scale=1.0
                                 ).then_inc(s_act, 1)
            act.wait_ge(s_red, n_chunks)
            nc.scalar.activation(ot[:, mid_r:], rt[:, mid_r:],
                                 mybir.ActivationFunctionType.Tanh,
                                 bias=bt[:, 0:1], scale=1.0
                                 ).then_inc(s_act, 1)

        @block.gpsimd
        def _(gps):
            for i, (n, off, ri) in enumerate(chunks):
                if i == 0:
                    gps.wait_ge(s_w, 16)
                if i >= 2:
                    gps.wait_ge(s_red, i - 1)
                gps.wait_ge(s_slot[i % bufs], 16 * (i // bufs + 1))
                x3, _ = x_view(i)
                pt = pts[i % 2]
                p3 = pt[:, :ri * C].rearrange("p (r c) -> p r c", c=C)
                wb = wt[:, :].unsqueeze(1).broadcast_to([P, ri, C])
                gps.tensor_mul(p3, x3, wb).then_inc(s_mul, 1)

        @block.vector
        def _(vec):
            vec.memset(bt[:, :], c0)
            for i, (n, off, ri) in enumerate(chunks):
                vec.wait_ge(s_mul, i + 1)
                pt = pts[i % 2]
                p3 = pt[:, :ri * C].rearrange("p (r c) -> p r c", c=C)
                col = n * r_max + off
                nc.vector.reduce_sum(rt[:, col:col + ri], p3,
                                     axis=mybir.AxisListType.X
                                     ).then_inc(s_red, 1)

    nc.compile()
    return nc


def _build_v9(b_core: int, c0: float, bufs: int = 3,
              head=(32, 96, 128), tail=(128, 128),
              r_max: int = 256) -> bass.Bass:
    """All-DVE compute at the 4x_2p rate: the weight multiply and a binary
    add-tree (17/8/4/2/1 + leftover) are all TensorScalarPtr ops on packed
    fp16 SBUF tiles, which DVE executes at 4 elem/cycle/lane (vs 1 for
    TENSOR_REDUCE, which has no fast mode).  Intermediates are consumed
    in program order on the one engine, so they are single-buffered.
    Sync/Scalar queues alternate the input loads; ACT does two batched
    tanh's; GpSimd idle."""
    from contextlib import ExitStack

    nc = bacc.Bacc()
    x = nc.declare_dram_parameter("x", [b_core, C], mybir.dt.float16,
                                  isOutput=False)
    w = nc.declare_dram_parameter("w", [P, C], mybir.dt.float16,
                                  isOutput=False)
    total_r = b_core // P
    y = nc.declare_dram_parameter("y", [P, total_r], mybir.dt.float16,
                                  isOutput=True)

    n_blocks = total_r // r_max
    assert total_r % r_max == 0
    assert sum(head) == r_max and sum(tail) == r_max

    chunks = []
    for off, ri in zip(np.cumsum((0,) + head[:-1]), head):
        chunks.append((0, int(off), ri))
    for n in range(1, n_blocks - 1):
        chunks.append((n, 0, r_max))
    for off, ri in zip(np.cumsum((0,) + tail[:-1]), tail):
        chunks.append((n_blocks - 1, int(off), ri))
    n_chunks = len(chunks)
    n_premid = len(head) + (n_blocks - 2)  # chunks covering blocks 0..n-2

    xb = x[:].rearrange("(n p r) s -> n p r s", p=P, r=r_max)
    mid_r = (n_blocks - 1) * r_max
    mult, add = mybir.AluOpType.mult, mybir.AluOpType.add
    f16 = mybir.dt.float16

    with ExitStack() as ctx:
        ef = ctx.enter_context
        xs = [ef(nc.sbuf_tensor(f"xs{k}", [P, r_max * C], f16))
              for k in range(bufs)]
        xm = ef(nc.sbuf_tensor("xm", [P, r_max * C], f16))
        t1 = ef(nc.sbuf_tensor("t1", [P, r_max * 17], f16))
        t2 = ef(nc.sbuf_tensor("t2", [P, r_max * 8], f16))
        t3 = ef(nc.sbuf_tensor("t3", [P, r_max * 4], f16))
        t4 = ef(nc.sbuf_tensor("t4", [P, r_max * 2], f16))
        t5 = ef(nc.sbuf_tensor("t5", [P, r_max], f16))
        rt = ef(nc.sbuf_tensor("rt", [P, total_r], f16))
        ot = ef(nc.sbuf_tensor("ot", [P, total_r], f16))
        wt = ef(nc.sbuf_tensor("wt", [P, C], f16))
        bt = ef(nc.sbuf_tensor("bt", [P, 1], mybir.dt.float32))
        s_slot = [ef(nc.semaphore(f"s_slot{k}")) for k in range(bufs)]
        s_w = ef(nc.semaphore("s_w"))
        s_dve = ef(nc.semaphore("s_dve"))
        s_act = ef(nc.semaphore("s_act"))
        s_out = ef(nc.semaphore("s_out"))
        block = ef(nc.Block())

        def x_view(i):
            n, off, ri = chunks[i]
            slot = xs[i % bufs]
            return (slot[:, :ri * C].rearrange("p (r s) -> p r s", s=C),
                    xb[n][:, off:off + ri, :])

        @block.sync
        def _(sync):
            sync.dma_start(wt[:, :], w[:]).then_inc(s_w, 16)
            for i in range(0, n_chunks, 2):
                if i - bufs >= 0:
                    sync.wait_ge(s_dve, i - bufs + 1)
                dst, src = x_view(i)
                sync.dma_start(dst, src).then_inc(s_slot[i % bufs], 16)
            sync.wait_ge(s_act, 1)
            sync.dma_start(y[:, :mid_r], ot[:, :mid_r]).then_inc(s_out, 16)
            sync.wait_ge(s_act, 2)
            sync.dma_start(y[:, mid_r:], ot[:, mid_r:]).then_inc(s_out, 16)
            sync.wait_ge(s_out, 32)

        @block.scalar
        def _(act):
            for i in range(1, n_chunks, 2):
                if i - bufs >= 0:
                    act.wait_ge(s_dve, i - bufs + 1)
                dst, src = x_view(i)
                act.dma_start(dst, src).then_inc(s_slot[i % bufs], 16)
            act.wait_ge(s_dve, n_premid)
            nc.scalar.activation(ot[:, :mid_r], rt[:, :mid_r],
                                 mybir.ActivationFunctionType.Tanh,
                                 bias=bt[:, 0:1], scale=1.0
                                 ).then_inc(s_act, 1)
            act.wait_ge(s_dve, n_chunks)
            nc.scalar.activation(ot[:, mid_r:], rt[:, mid_r:],
                                 mybir.ActivationFunctionType.Tanh,
                                 bias=bt[:, 0:1], scale=1.0
                                 ).then_inc(s_act, 1)

        @block.vector
        def _(vec):
            vec.memset(bt[:, :], c0)
            for i, (n, off, ri) in enumerate(chunks):
                if i == 0:
                    vec.wait_ge(s_w, 16)
                vec.wait_ge(s_slot[i % bufs], 16 * (i // bufs + 1))
                x3, _ = x_view(i)
                m3 = xm[:, :ri * C].rearrange("p (r c) -> p r c", c=C)
                wb = wt[:, :].unsqueeze(1).broadcast_to([P, ri, C])
                vec.scalar_tensor_tensor(m3, x3, 1.0, wb, mult, mult)
                v1 = t1[:, :ri * 17].rearrange("p (r c) -> p r c", c=17)
                vec.scalar_tensor_tensor(v1, m3[:, :, 0:17], 1.0,
                                         m3[:, :, 17:34], mult, add)
                v2 = t2[:, :ri * 8].rearrange("p (r c) -> p r c", c=8)
                vec.scalar_tensor_tensor(v2, v1[:, :, 0:8], 1.0,
                                         v1[:, :, 8:16], mult, add)
                v3 = t3[:, :ri * 4].rearrange("p (r c) -> p r c", c=4)
                vec.scalar_tensor_tensor(v3, v2[:, :, 0:4], 1.0,
                                         v2[:, :, 4:8], mult, add)
                v5 = t5[:, :ri]
                with nc.allow_low_precision(reason="fp16 dot, 2e-2 tol"):
                    vec.reduce_sum(v5, v3, axis=mybir.AxisListType.X)
                col = n * r_max + off
                vec.scalar_tensor_tensor(rt[:, col:col + ri], v5, 1.0,
                                         v1[:, :, 16], mult, add
                                         ).then_inc(s_dve, 1)

    nc.compile()
    return nc


def _build_v10(b_core: int, c0: float, bufs: int = 4,
               head=(96, 160), tail=(160, 96),
               r_max: int = 256) -> bass.Bass:
    """v9 + two-chunk software pipelining on DVE.  Consecutive DVE ops in
    one chunk's mul/add-tree chain are RAW-dependent, and the DVE pipeline
    does not interlock SBUF reads against the previous op's in-flight
    write tail, so dependent ops back-to-back intermittently read stale
    data.  Interleaving two chunks (a/b tile sets) puts one full
    independent op between every dependent pair.  The 17-wide leftover
    column is copied (tt-max) into a 5-wide t3 so each chunk ends in one
    TENSOR_REDUCE with >=2 ops of separation from its producers."""
    from contextlib import ExitStack

    nc = bacc.Bacc()
    x = nc.declare_dram_parameter("x", [b_core, C], mybir.dt.float16,
                                  isOutput=False)
    w = nc.declare_dram_parameter("w", [P, C], mybir.dt.float16,
                                  isOutput=False)
    total_r = b_core // P
    y = nc.declare_dram_parameter("y", [P, total_r], mybir.dt.float16,
                                  isOutput=True)

    n_blocks = total_r // r_max
    assert total_r % r_max == 0
    assert sum(head) == r_max and sum(tail) == r_max

    chunks = []
    for off, ri in zip(np.cumsum((0,) + head[:-1]), head):
        chunks.append((0, int(off), ri))
    for n in range(1, n_blocks - 1):
        chunks.append((n, 0, r_max))
    for off, ri in zip(np.cumsum((0,) + tail[:-1]), tail):
        chunks.append((n_blocks - 1, int(off), ri))
    n_chunks = len(chunks)
    assert n_chunks % 2 == 0
    n_premid = len(head) + (n_blocks - 2)  # chunks covering blocks 0..n-2

    xb = x[:].rearrange("(n p r) s -> n p r s", p=P, r=r_max)
    mid_r = (n_blocks - 1) * r_max
    mult, add = mybir.AluOpType.mult, mybir.AluOpType.add
    amax = mybir.AluOpType.max
    f16 = mybir.dt.float16

    with ExitStack() as ctx:
        ef = ctx.enter_context
        xs = [ef(nc.sbuf_tensor(f"xs{k}", [P, r_max * C], f16))
              for k in range(bufs)]
        xm = [ef(nc.sbuf_tensor(f"xm{k}", [P, r_max * C], f16))
              for k in range(2)]
        t1 = [ef(nc.sbuf_tensor(f"t1{k}", [P, r_max * 17], f16))
              for k in range(2)]
        t2 = [ef(nc.sbuf_tensor(f"t2{k}", [P, r_max * 8], f16))
              for k in range(2)]
        t3 = [ef(nc.sbuf_tensor(f"t3{k}", [P, r_max * 5], f16))
              for k in range(2)]
        rt = ef(nc.sbuf_tensor("rt", [P, total_r], f16))
        ot = ef(nc.sbuf_tensor("ot", [P, total_r], f16))
        wt = ef(nc.sbuf_tensor("wt", [P, C], f16))
        bt = ef(nc.sbuf_tensor("bt", [P, 1], mybir.dt.float32))
        s_slot = [ef(nc.semaphore(f"s_slot{k}")) for k in range(bufs)]
        s_w = ef(nc.semaphore("s_w"))
        s_x = ef(nc.semaphore("s_x"))      # chunk's xs slot consumed (mul)
        s_dve = ef(nc.semaphore("s_dve"))  # chunk's rt columns written
        s_act = ef(nc.semaphore("s_act"))
        s_out = ef(nc.semaphore("s_out"))
        block = ef(nc.Block())

        def x_view(i):
            n, off, ri = chunks[i]
            slot = xs[i % bufs]
            return (slot[:, :ri * C].rearrange("p (r s) -> p r s", s=C),
                    xb[n][:, off:off + ri, :])

        @block.sync
        def _(sync):
            sync.dma_start(wt[:, :], w[:]).then_inc(s_w, 16)
            for i in range(0, n_chunks, 2):
                if i - bufs >= 0:
                    sync.wait_ge(s_x, i - bufs + 1)
                dst, src = x_view(i)
                sync.dma_start(dst, src).then_inc(s_slot[i % bufs], 16)
            sync.wait_ge(s_act, 1)
            sync.dma_start(y[:, :mid_r], ot[:, :mid_r]).then_inc(s_out, 16)
            sync.wait_ge(s_act, 2)
            sync.dma_start(y[:, mid_r:], ot[:, mid_r:]).then_inc(s_out, 16)
            sync.wait_ge(s_out, 32)

        @block.scalar
        def _(act):
            for i in range(1, n_chunks, 2):
                if i - bufs >= 0:
                    act.wait_ge(s_x, i - bufs + 1)
                dst, src = x_view(i)
                act.dma_start(dst, src).then_inc(s_slot[i % bufs], 16)
            act.wait_ge(s_dve, n_premid)
            nc.scalar.activation(ot[:, :mid_r], rt[:, :mid_r],
                                 mybir.ActivationFunctionType.Tanh,
                                 bias=bt[:, 0:1], scale=1.0
                                 ).then_inc(s_act, 1)
            act.wait_ge(s_dve, n_chunks)
            nc.scalar.activation(ot[:, mid_r:], rt[:, mid_r:],
                                 mybir.ActivationFunctionType.Tanh,
                                 bias=bt[:, 0:1], scale=1.0
                                 ).then_inc(s_act, 1)

        @block.vector
        def _(vec):
            vec.memset(bt[:, :], c0)
            for i0 in range(0, n_chunks, 2):
                pair = (i0, i0 + 1)
                views = []
                for k, i in enumerate(pair):
                    n, off, ri = chunks[i]
                    x3, _ = x_view(i)
                    m3 = xm[k][:, :ri * C].rearrange("p (r c) -> p r c", c=C)
                    v1 = t1[k][:, :ri * 17].rearrange("p (r c) -> p r c",
                                                      c=17)
                    v2 = t2[k][:, :ri * 8].rearrange("p (r c) -> p r c", c=8)
                    v3 = t3[k][:, :ri * 5].rearrange("p (r c) -> p r c", c=5)
                    views.append((i, ri, n * r_max + off, x3, m3, v1, v2, v3))

                for i, ri, col, x3, m3, v1, v2, v3 in views:
                    if i == 0:
                        vec.wait_ge(s_w, 16)
                    vec.wait_ge(s_slot[i % bufs], 16 * (i // bufs + 1))
                    wb = wt[:, :].unsqueeze(1).broadcast_to([P, ri, C])
                    vec.scalar_tensor_tensor(m3, x3, 1.0, wb, mult, mult
                                             ).then_inc(s_x, 1)
                for i, ri, col, x3, m3, v1, v2, v3 in views:
                    vec.scalar_tensor_tensor(v1, m3[:, :, 0:17], 1.0,
                                             m3[:, :, 17:34], mult, add)
                for i, ri, col, x3, m3, v1, v2, v3 in views:
                    vec.scalar_tensor_tensor(v2, v1[:, :, 0:8], 1.0,
                                             v1[:, :, 8:16], mult, add)
                for i, ri, col, x3, m3, v1, v2, v3 in views:
                    vec.tensor_copy(v3[:, :, 4], v1[:, :, 16])
                for i, ri, col, x3, m3, v1, v2, v3 in views:
                    vec.scalar_tensor_tensor(v3[:, :, 0:4], v2[:, :, 0:4],
                                             1.0, v2[:, :, 4:8], mult, add)
                for i, ri, col, x3, m3, v1, v2, v3 in views:
                    with nc.allow_low_precision(reason="fp16 dot, 2e-2 tol"):
                        vec.reduce_sum(rt[:, col:col + ri], v3,
                                       axis=mybir.AxisListType.X
                                       ).then_inc(s_dve, 1)

    nc.compile()
    return nc


def _build(b_core: int, c0: float, r: int = R, bufs: int = 3,
           sliced: bool = False, alt_queues: bool = True) -> bass.Bass:
    nc = bacc.Bacc()
    x = nc.declare_dram_parameter("x", [b_core, S], mybir.dt.float32,
                                  isOutput=False)
    w = nc.declare_dram_parameter("w", [P, C], mybir.dt.float32,
                                  isOutput=False)
    y = nc.declare_dram_parameter("y", [b_core], mybir.dt.float32,
                                  isOutput=True)

    rows_per_tile = P * r
    n_tiles = b_core // rows_per_tile
    assert b_core % rows_per_tile == 0

    xv = x[:].rearrange("(n p r) s -> n p r s", p=P, r=r)
    yv = y[:].rearrange("(n p r) -> n p r", p=P, r=r)

    with tile.TileContext(nc) as tc:
        with (
            tc.tile_pool(name="wp", bufs=1) as wp,
            tc.tile_pool(name="xp", bufs=bufs) as xp,
            tc.tile_pool(name="pp", bufs=2) as pp,
            tc.tile_pool(name="rp", bufs=2) as rp,
            tc.tile_pool(name="op", bufs=2) as op,
        ):
            wt = wp.tile([P, C], mybir.dt.float32)
            nc.sync.dma_start(wt[:], w[:])
            bt = wp.tile([P, 1], mybir.dt.float32, tag="bias")
            nc.vector.memset(bt[:], c0)
            for i in range(n_tiles):
                dma_eng = nc.scalar if (alt_queues and i % 2) else nc.sync
                if sliced:
                    xt = xp.tile([P, r * C], mybir.dt.float32)
                    x3 = xt[:].rearrange("p (r c) -> p r c", c=C)
                    dma_eng.dma_start(x3, xv[i][:, :, C_LO:C_HI])
                else:
                    xt = xp.tile([P, r * S], mybir.dt.float32)
                    x3full = xt[:].rearrange("p (r s) -> p r s", s=S)
                    dma_eng.dma_start(x3full, xv[i])
                    x3 = x3full[:, :, C_LO:C_HI]
                pt = pp.tile([P, r * C], mybir.dt.float32)
                p3 = pt[:].rearrange("p (r c) -> p r c", c=C)
                wb = wt[:].unsqueeze(1).broadcast_to([P, r, C])
                nc.vector.tensor_mul(p3, x3, wb)
                rt = rp.tile([P, r], mybir.dt.float32)
                nc.vector.reduce_sum(rt[:], p3, axis=mybir.AxisListType.X)
                ot = op.tile([P, r], mybir.dt.float32)
                nc.scalar.activation(ot[:], rt[:],
                                     mybir.ActivationFunctionType.Tanh,
                                     bias=bt[:, 0:1], scale=1.0)
                nc.sync.dma_start(yv[i], ot[:])
    nc.compile()
    return nc


def _prepare_v8(inputs):
    """Host-side prep shared by kernel() and the trace harness: collapsed
    weights, fp16 pre-sliced input, per-core in_maps, and the nc."""
    price = np.asarray(inputs["price_series"])
    B = price.shape[0]
    assert B % N_CORES == 0
    b_core = B // N_CORES

    u, c0 = _collapsed_weights(
        inputs["w_fast"], inputs["b_fast"], inputs["w_slow"],
        inputs["b_slow"], inputs["w_sig"], inputs["b_sig"],
        inputs["norm_scale"], inputs["norm_bias"])
    xs = np.ascontiguousarray(price[:, C_LO:C_HI]).astype(np.float16)
    w16 = np.ascontiguousarray(
        np.broadcast_to(u[C_LO:C_HI].astype(np.float16)[None, :], (P, C)))
    nc = _build_v10(b_core, c0)
    in_maps = [
        {"x": xs[i * b_core:(i + 1) * b_core], "w": w16}
        for i in range(N_CORES)
    ]
    return nc, in_maps, b_core


R_MAX = 256  # rows-per-partition per block in the active build


def _gather_v8(res, b_core: int) -> np.ndarray:
    """y_dev[p, n*R_MAX + r] holds the output for row (n*P + p)*R_MAX + r."""
    outs = []
    for i in range(N_CORES):
        yd = np.asarray(res.results[i]["y"]).reshape(P, b_core // P)
        yd = yd.reshape(P, -1, R_MAX).transpose(1, 0, 2).reshape(-1)
        outs.append(yd.astype(np.float32))
    return np.concatenate(outs)


def kernel(**inputs) -> np.ndarray:
    price = np.asarray(inputs["price_series"])
    B = price.shape[0]
    assert B % N_CORES == 0
    b_core = B // N_CORES

    if b_core % (P * R_MAX) == 0 and b_core // (P * R_MAX) >= 2:
        nc, in_maps, b_core = _prepare_v8(inputs)
        res = run_bass_kernel_spmd(nc, in_maps, list(range(N_CORES)))
        return _gather_v8(res, b_core).reshape(B, 1)

    price = np.ascontiguousarray(np.asarray(price, dtype=np.float32))
    u, c0 = _collapsed_weights(
        inputs["w_fast"], inputs["b_fast"], inputs["w_slow"],
        inputs["b_slow"], inputs["w_sig"], inputs["b_sig"],
        inputs["norm_scale"], inputs["norm_bias"])
    w_rep = np.ascontiguousarray(
        np.broadcast_to(u[C_LO:C_HI][None, :], (P, C)))
    nc = _build(b_core, c0, r=max(1, min(64, b_core // P)))
    in_maps = [
        {"x": price[i * b_core:(i + 1) * b_core], "w": w_rep}
        for i in range(N_CORES)
    ]
    res = run_bass_kernel_spmd(nc, in_maps, list(range(N_CORES)))
    out = np.concatenate([res.results[i]["y"].reshape(-1)
                          for i in range(N_CORES)])
    return out.reshape(B, 1).astype(np.float32)

